# revision 25
# baseline (speedup 1.0000x reference)
"""CP-factorized embedding lookup on 8 TRN2 NeuronCores.

Reference computes full[a,b,c,d,e,f] = sum_r U0[a,r]*...*U5[f,r], reshapes to a
(50000, 512) table, and gathers rows by x. We never materialize the table:

  out[n, e] = sum_r (U0[a_n,r]*U1[b_n,r]*U2[c_n,r]) * (U3[d,r]*U4[e2,r]*U5[f,r])
            = sum_r V[n, r] * W[e, r]

with v = 1000a + 25b + c and e = 64d + 8e2 + f.

Per core (1024 indices, data-parallel over the 8192 total), in two pipelined
512-index halves:
  1. broadcast x across 115 partitions (50+40+25 stacked factor rows) and
     decompose it in place with per-partition constants in a short 16-bit
     DVE chain (4x perf mode):
       rows  0:50  -> a      = floor(v/1000)   (1000 when v == 0: see below)
       rows 50:90  -> b + 50 = floor(v/25) - 40*floor(v/1000) + 50
       rows 90:115 -> c + 90 = (v-25000) - 25*(floor(v/25)-1000) + 90
     floor(v/d) = f32->i16 cast of (v + bias)*(1/d); the HW cast rounds to
     nearest even, bias = -(d/2 - 0.5) puts the value mid-interval, so the
     result is exact. Block 2 is offset by -25000 to fit int16. The
     padding mask is folded in: rows 0:50 use s2 = min(v, 1) and
     diff = a - 1000*s2 + 1000, which equals a for v > 0 and 1000 (no
     one-hot hit -> zero row) for v == 0.
  2. one-hot[115, 512] = is_equal(diff, iota); gather via one PE matmul
     with block-diag stacked [U0;U1;U2] as lhsT -> psum[96, 512];
     V = elementwise product of the three 32-row blocks
  3. W[32, 512] = Khatri-Rao of U3,U4,U5 built with two broadcast multiplies
     (U3/U4/U5 transposed on-chip through the PE)
  4. out chunk c: matmul(lhsT=V[:,128j:128j+128], rhs=W) -> psum, two chunks
     batched per [128, 1024] psum pair, one Scalar-engine copy -> SBUF,
     one DMA per 256 output rows

All small constant operands (decomposition table, iota, identity, stacked
U3..U5, block-diagonal [U0;U1;U2]) are packed host-side into one aux input
(pure rearrangement/zero-padding -- all arithmetic stays on device) so the
front end costs a single small DMA. Matmul operands are produced as float32r
(tf32-like, 1 row/cycle vs 4 for float32); one-hot entries are exact in any
dtype and the factor rounding error is ~1e-4 relative, far inside tolerance.
"""

import numpy as np

import concourse.bass as bass
import concourse.mybir as mybir
import concourse.tile as tile
from concourse import bacc
from concourse.bass_utils import run_bass_kernel_spmd

F32 = mybir.dt.float32
F32R = mybir.dt.float32r
I32 = mybir.dt.int32
I16 = mybir.dt.int16
U16 = mybir.dt.uint16
ALU = mybir.AluOpType

N_CORES = 8
PER_CORE = 1024           # indices per core (8192 / 8)
HALF = 512                # pipeline granularity (one PSUM bank of columns)
EMB = 512
RANK = 32
KV = 115                  # 50 + 40 + 25 stacked vocab-factor rows
MV = 96                   # 3 * RANK stacked outputs

R1000 = float(np.float32(1.0 / 1000.0))
R25 = float(np.float32(1.0 / 25.0))

# aux layout: [115, 7 + 24 + 32 + 96]
CC_OFF = 0      # [115, 7] decomposition constants + iota
ID_OFF = 7      # [24, 24] identity (rows 0:24)
U345_OFF = 31   # [24, 32] stacked U3;U4;U5 (rows 0:24)
UBLK_OFF = 63   # [115, 96] block-diag [U0;U1;U2]
ONES_OFF = 159  # [1, 115] row of ones (lhsT of the broadcast matmul)
AUX_W = 274

# matmul operand dtype: float32r streams 1 row/cycle (vs 4 for float32).
MM_DT = F32R


def _const_table() -> np.ndarray:
    """[115, 7] per-partition constants: b1, R1, b2, R2, K, OFF, iota.

    Chain (s1, s2 are f32->i16 floor stages; the cast rounds to nearest):
      s1 = rint((v + b1) * R1);  s2 = rint((v + b2) * R2)
      (rows 0:50 overwrite: s2 = min(v, 1))
      diff = s1 - (K*s2 - OFF)  ; onehot = (diff == iota)
    """
    cc = np.zeros((KV, 7), np.float32)
    rows = ((0, 50), (50, 90), (90, 115))
    vals = [
        # s1 = a; s2 = min(v,1); hit iff a == 1000*s2 - 1000 + p
        (-499.5, R1000, 0.0, 1.0, 1000.0, 1000.0),
        # s1 = q25; s2 = a; hit iff q25 == 40a - 50 + p  (p abs. row 50..89)
        (-12.0, R25, -499.5, R1000, 40.0, 50.0),
        # s1 = v-25000; s2 = q25-1000; hit iff s1 == 25*s2 - 90 + p
        (-25000.0, 1.0, -25012.0, R25, 25.0, 90.0),
    ]
    for (lo, hi), v6 in zip(rows, vals):
        cc[lo:hi, 0:6] = np.float32(v6)
    # OFF2 = OFF - row: tkp = K*s2 - OFF2 and the one-hot becomes a single
    # fused tensor_tensor is_equal(s1, tkp)
    cc[:, 5] -= np.arange(KV, dtype=np.float32)
    return cc


def _aux_table(us: list[np.ndarray]) -> np.ndarray:
    aux = np.zeros((KV, AUX_W), np.float32)
    aux[:, CC_OFF:CC_OFF + 7] = _const_table()
    aux[0:24, ID_OFF:ID_OFF + 24] = np.eye(24, dtype=np.float32)
    aux[0:8, U345_OFF:U345_OFF + 32] = us[3]
    aux[8:16, U345_OFF:U345_OFF + 32] = us[4]
    aux[16:24, U345_OFF:U345_OFF + 32] = us[5]
    aux[0:50, UBLK_OFF:UBLK_OFF + 32] = us[0]
    aux[50:90, UBLK_OFF + 32:UBLK_OFF + 64] = us[1]
    aux[90:115, UBLK_OFF + 64:UBLK_OFF + 96] = us[2]
    aux[0, ONES_OFF:ONES_OFF + KV] = 1.0
    return aux


def build():
    nc = bacc.Bacc("TRN2", target_bir_lowering=False, debug=False)

    x = nc.dram_tensor("x", [PER_CORE], I32, kind="ExternalInput")
    aux_d = nc.dram_tensor("aux", [KV, AUX_W], F32, kind="ExternalInput")
    out = nc.dram_tensor("out", [PER_CORE, EMB], F32, kind="ExternalOutput")

    NH = PER_CORE // HALF   # 2 halves
    NC2 = HALF // 256       # 2 two-chunk groups per half

    with tile.TileContext(nc) as tc:
        with (
            tc.tile_pool(name="const", bufs=1) as cpool,
            tc.tile_pool(name="work", bufs=2) as wpool,
            tc.tile_pool(name="vpsum", bufs=2, space="PSUM") as ppool,
            tc.tile_pool(name="osb", bufs=2) as opool,
            tc.tile_pool(name="opsum", bufs=2, space="PSUM") as oppool,
        ):
            # ---- broadcast x across the 115 stacked factor rows (one
            # full-width DMA on the sync ring); aux lands in parallel on
            # the scalar ring.
            aux = cpool.tile([KV, AUX_W], F32)
            nc.scalar.dma_start(out=aux[:], in_=aux_d[:])
            xrep = cpool.tile([KV, PER_CORE], I32)
            for si, (lo, hi) in enumerate(((0, 29), (29, 58), (58, 87), (87, KV))):
                eng = nc.sync if si % 2 == 0 else nc.scalar
                eng.dma_start(
                    out=xrep[lo:hi, :],
                    in_=x[:].unsqueeze(0).partition_broadcast(hi - lo),
                )
            cc = aux[:, CC_OFF:CC_OFF + 7]
            idm = aux[0:24, ID_OFF:ID_OFF + 24]
            u345 = aux[0:24, U345_OFF:U345_OFF + 32]

            # f32r-rounded copy of the block-diag factors for the gather mm
            ublk = cpool.tile([KV, MV], MM_DT)
            nc.vector.tensor_copy(out=ublk[:], in_=aux[:, UBLK_OFF:UBLK_OFF + 96])

            # ---- PE warm-up: ~3.5us of junk matmuls so HAM reaches full
            # clock before the real matmuls issue
            wps = ppool.tile([MV, MV], F32, tag="warm", bufs=1)
            for w in range(24):
                nc.tensor.matmul(
                    wps[:], lhsT=ublk[:], rhs=ublk[:], start=True, stop=True
                )
            wjunk = cpool.tile([1, 8], F32)
            nc.scalar.copy(out=wjunk[:], in_=wps[0:1, 0:8])

            # ---- W[r, e] = U3[d,r] * U4[e2,r] * U5[f,r],  e = 64d + 8e2 + f
            u345t_ps = ppool.tile([RANK, 24], F32, tag="pv")
            nc.tensor.transpose(u345t_ps[:], u345, idm)
            u345t = cpool.tile([RANK, 24], F32)
            nc.scalar.copy(out=u345t[:], in_=u345t_ps[:])
            t45 = cpool.tile([RANK, 64], F32)
            nc.vector.tensor_tensor(
                out=t45[:].rearrange("r (e f) -> r e f", e=8),
                in0=u345t[:, 8:16].unsqueeze(2).broadcast_to([RANK, 8, 8]),
                in1=u345t[:, 16:24].unsqueeze(1).broadcast_to([RANK, 8, 8]),
                op=ALU.mult,
            )
            wt = cpool.tile([RANK, EMB], MM_DT)
            nc.vector.tensor_tensor(
                out=wt[:].rearrange("r (d ef) -> r d ef", d=8),
                in0=u345t[:, 0:8].unsqueeze(2).broadcast_to([RANK, 8, 64]),
                in1=t45[:].unsqueeze(1).broadcast_to([RANK, 8, 64]),
                op=ALU.mult,
            )

            # ---- full-width 5-op decomposition chain straight off the
            # int32 broadcast (mixed int-in/f32-scalar tensor_scalar is
            # exact on HW: internal fp32 ALU + round-to-nearest int cast)
            s1 = cpool.tile([KV, PER_CORE], I16)
            nc.vector.tensor_scalar(
                out=s1[:], in0=xrep[:], scalar1=cc[:, 0:1], scalar2=cc[:, 1:2],
                op0=ALU.add, op1=ALU.mult,
            )
            s2 = cpool.tile([KV, PER_CORE], I16)
            nc.vector.tensor_scalar(
                out=s2[:], in0=xrep[:], scalar1=cc[:, 2:3], scalar2=cc[:, 3:4],
                op0=ALU.add, op1=ALU.mult,
            )
            # rows 0:50: s2 = min(v, 1) -> folds the v==0 padding mask into
            # the block-0 one-hot (no hit for v == 0 -> zero output row)
            nc.vector.tensor_scalar(
                out=s2[0:50, :], in0=xrep[0:50, :], scalar1=1.0, scalar2=1.0,
                op0=ALU.min, op1=ALU.mult,
            )
            tkp = cpool.tile([KV, PER_CORE], I16)
            nc.vector.tensor_scalar(
                out=tkp[:], in0=s2[:], scalar1=cc[:, 4:5], scalar2=cc[:, 5:6],
                op0=ALU.mult, op1=ALU.subtract,
            )
            onehot = cpool.tile([KV, PER_CORE], MM_DT)
            nc.vector.tensor_tensor(
                out=onehot[:], in0=s1[:], in1=tkp[:], op=ALU.is_equal
            )

            for h in range(NH):
                pv = ppool.tile([MV, HALF], F32, name=f"pv_{h}", tag="pv")
                nc.tensor.matmul(
                    pv[:], lhsT=ublk[:],
                    rhs=onehot[:, h * HALF:(h + 1) * HALF],
                    start=True, stop=True,
                )
                # DVE may read only one PSUM operand per op: stage block 0
                # to SBUF on the Scalar engine first.
                s0 = wpool.tile([RANK, HALF], F32, name=f"s0_{h}", tag="s0")
                nc.scalar.copy(out=s0[:], in_=pv[0:32, :])
                v01 = wpool.tile([RANK, HALF], F32, name=f"v01_{h}", tag="v01")
                nc.vector.tensor_tensor(
                    out=v01[:], in0=s0[:], in1=pv[32:64, :], op=ALU.mult
                )
                vth = cpool.tile([RANK, HALF], MM_DT, name=f"vt_{h}")
                nc.vector.tensor_tensor(
                    out=vth[:], in0=v01[:], in1=pv[64:96, :], op=ALU.mult
                )

                # two output chunks batched per [128, 1024] psum pair
                for g in range(NC2):
                    po2 = oppool.tile([128, 2 * EMB], F32, name=f"po_{h}{g}",
                                      tag="po")
                    for j in range(2):
                        nc.tensor.matmul(
                            po2[:, j * EMB:(j + 1) * EMB],
                            lhsT=vth[:, (2 * g + j) * 128:(2 * g + j + 1) * 128],
                            rhs=wt[:],
                            start=True, stop=True,
                        )
                    osb = opool.tile([128, 2 * EMB], F32, name=f"osb_{h}{g}",
                                     tag="osb")
                    if g == 0:
                        nc.scalar.copy(out=osb[:], in_=po2[:])
                    else:
                        nc.vector.tensor_copy(out=osb[:], in_=po2[:])
                    row0 = h * HALF + g * 256
                    nc.sync.dma_start(
                        out=out[row0:row0 + 256, :].rearrange(
                            "(j p) e -> p j e", p=128
                        ),
                        in_=osb[:].rearrange("p (j e) -> p j e", j=2),
                    )

    nc.compile()
    return nc


_CACHE: dict = {}


def _get_nc():
    if "nc" not in _CACHE:
        _CACHE["nc"] = build()
    return _CACHE["nc"]


def run(inputs, **spmd_kwargs):
    nc = _get_nc()
    x = np.ascontiguousarray(inputs["x"].reshape(-1), dtype=np.int32)
    us = [
        np.ascontiguousarray(inputs[f"U{j}"], dtype=np.float32) for j in range(6)
    ]
    aux = _aux_table(us)
    in_maps = []
    for i in range(N_CORES):
        in_maps.append({"x": x[i * PER_CORE:(i + 1) * PER_CORE], "aux": aux})
    res = run_bass_kernel_spmd(
        nc, in_maps, core_ids=list(range(N_CORES)), **spmd_kwargs
    )
    shards = [np.asarray(res.results[i]["out"]) for i in range(N_CORES)]
    full = np.concatenate(shards, axis=0).reshape(4, 2048, EMB)
    return full.astype(np.float32, copy=False), res


def kernel(**inputs) -> np.ndarray:
    return run(inputs)[0]
